# revision 8
# baseline (speedup 1.0000x reference)
"""LSTM cell (batch 8192, input 512, hidden 512) on 8 Trainium2 NeuronCores.

Data-parallel over the batch dim: each core handles 1024 rows; weights are
replicated. Everything is computed in [hidden, batch] layout with the
contraction dim (fan_in = 1024) on SBUF partitions:

  gate.T[n, b] = sum_k W.T[k, n] * combined.T[k, b]     (matmul: lhsT.T @ rhs)

Precision plan (measured rel-err ~1.55e-2 vs the 2e-2 budget): the i-gate
matmul runs in fp8-e4m3 with MatmulPerfMode.DoubleRow (K=256 per
instruction at the same ~216ns fill as a K=128 bf16 matmul = 2x MACs),
weights pre-scaled by 128 and descaled via the ACT scale operand. The
f/c/o gates stay bf16: their error amplification (f multiplies c_prev,
c~ passes tanh' = 1, o hits h directly) makes fp8 too lossy for them.
Gates are f32; cn/th/hn and both outputs are bf16; c_prev ships bf16.

Schedule facts this version is built around (measured on HW + trace):
- Matmul fill is N columns at 2.4GHz regardless of dtype: 512-col matmul
  = ~216ns back-to-back. PE floor = 224 matmul-equivalents x 216 ~ 48.4us.
- PE clock ramps to full over ~4-6us of near-continuous activity (sub-us
  gaps don't reset it); warmup fp8-DR dummies run straight into phase 1.
- DMA per-queue throughput is PACKET-CADENCE bound (~160-400ns/packet,
  packets = contiguous runs capped at 4KB). Tensors with short rows
  (<2KB) crawl at <100GB/s; >=4KB rows hit ~350GB/s. So w8i ships as ONE
  [128,4096] tensor (sync ring) while a8/a16/cp chunk into 256KB pieces
  with 2KB rows (gpsimd ring) for early semaphores at decent rate.
- Phase 1 (i-gate, fp8) runs kp-major over h={0,1,2} x b2 (6 PSUM banks)
  then h=3 from SBUF on 2 more banks; the ramping PE (~430ns/mm) trails
  the a8 chunk arrivals (~1us apart), so no stalls.
- The measured window ends after the LAST output DMA packet + drains, so
  the final h-strip computes c for both halves, then f for b2=1 BEFORE
  b2=0 (so b2=1's cn/tanh chain clears the scalar engine early), then o
  for b2=0, then o for b2=1 in column halves: after the very last matmul
  only ACT(o half) + mul + DMA remain. Final chunks fan out across the
  gpsimd/scalar/sync rings to overlap the ~1us DGE latency.
- ~10.8us of the measured window is fixed framework pre/postamble (const
  memsets, engine preambles, 254 semaphore clears at exit) - not
  reachable from kernel code.
"""

import numpy as np

import concourse.bacc as bacc
import concourse.bass as bass
import concourse.mybir as mybir
from concourse import tile
from concourse.bass_utils import run_bass_kernel_spmd

N_CORES = 8
BATCH = 8192
B = BATCH // N_CORES  # 1024 batch rows per core
K = 1024              # fan_in = input_dim + hidden_dim
H = 512               # hidden dim
KT = K // 128         # 8 bf16 contraction tiles
KP = K // 256         # 4 fp8 DoubleRow contraction tiles
HT = H // 128         # 4 hidden chunks per gate
BT = B // 512         # 2 batch halves (PSUM free-dim limit is 512 f32)
SW = 128.0            # fp8 weight pre-scale (descaled in ACT)
NWARM = 5             # fp8-DR warmup matmuls (cover preamble->w8i lands)

E4 = mybir.dt.float8e4
BF = mybir.dt.bfloat16
F32 = mybir.dt.float32
DR = mybir.MatmulPerfMode.DoubleRow

_SIG = mybir.ActivationFunctionType.Sigmoid
_TANH = mybir.ActivationFunctionType.Tanh


def _build():
    nc = bacc.Bacc(
        "TRN2",
        target_bir_lowering=False,
        debug=False,
        num_devices=N_CORES,
    )

    # a8_{kp} rows p, cols j*B + b  (j = which 128-half of the 256 k-block)
    a8_d = [nc.dram_tensor(f"a8_{kp}", [128, 2 * B], E4, kind="ExternalInput")
            for kp in range(KP)]
    # w8i rows p, cols kp*1024 + h*256 + j*128 + m  (4KB rows -> fast DMA)
    w8i = nc.dram_tensor("w8i", [128, KP * HT * 256], E4, kind="ExternalInput")
    # a16 rows k*128+p, cols b  ([128,B] slices are contiguous 256KB)
    a16 = nc.dram_tensor("a16", [K, B], BF, kind="ExternalInput")
    # w16h0 rows p, cols k*384 + gi*128 + m  (gi: 0=c, 1=f, 2=o; 6KB rows)
    w16h0 = nc.dram_tensor("w16h0", [128, KT * 384], BF, kind="ExternalInput")
    # w16h_{h} rows p, cols k*384 + gi*128 + m
    w16h_d = [None] + [
        nc.dram_tensor(f"w16h_{h}", [128, KT * 384], BF, kind="ExternalInput")
        for h in range(1, HT)]
    # bias2d col = g*HT + h, gate order (i, c, f, o)
    bias2d = nc.dram_tensor("bias2d", [128, 4 * HT], F32, kind="ExternalInput")
    cp16 = nc.dram_tensor("cp16", [H, B], BF, kind="ExternalInput")
    h_nextT = nc.dram_tensor("h_nextT", [H, B], BF, kind="ExternalOutput")
    c_nextT = nc.dram_tensor("c_nextT", [H, B], BF, kind="ExternalOutput")

    with tile.TileContext(nc) as tc:
        with (
            tc.tile_pool(name="acts", bufs=1) as apool,
            tc.tile_pool(name="wts", bufs=1) as wpool,
            tc.tile_pool(name="igates", bufs=1) as ipool,
            tc.tile_pool(name="gates", bufs=3) as gpool,
            tc.tile_pool(name="ew", bufs=3) as epool,
            tc.tile_pool(name="psum", bufs=1, space="PSUM") as pspool,
        ):
            # --- PE clock warm-up ----------------------------------------
            # fp8-DR dummies over a memset tile while the first input DMAs
            # are in flight; same dtype as phase 1, sized to end right as
            # w8i lands so the PE never idles (idle resets the DVFS ramp).
            warm_t = wpool.tile([128, 2, 512], E4, tag="warm", name="warm")
            nc.vector.memset(warm_t[:], 0.0)
            ps_warm = pspool.tile([128, 512], F32, tag="psC1", name="ps_warm")
            for r in range(NWARM):
                nc.tensor.matmul(
                    ps_warm[:], warm_t[:, :, 0:128], warm_t[:],
                    start=(r == 0), stop=(r == NWARM - 1),
                    perf_mode=DR,
                )
            warm_o = wpool.tile([128, 512], F32, tag="warm_o", name="warm_o")
            nc.vector.tensor_scalar_mul(warm_o[:], ps_warm[:], 0.0)

            # --- input DMA: two rings, exact need order -------------------
            # sync: bias, w8i, then the four fat w16 strips (all 4-6KB
            # rows -> full-rate); gpsimd: the 256KB activation chunks
            # (2KB rows) whose early semaphores pace phases 1 and 2.
            bias_t = wpool.tile([128, 4 * HT], F32, tag="bias", name="bias")
            nc.sync.dma_start(bias_t[:], bias2d[:])

            w8_t = wpool.tile([128, KP, HT, 2, 128], E4, tag="w8", name="w8")
            nc.sync.dma_start(w8_t[:], w8i[:])

            w16h0_t = wpool.tile([128, KT, 3, 128], BF, tag="w16h0", name="w16h0")
            nc.sync.dma_start(w16h0_t[:], w16h0[:])
            w16h_t = [None] * HT
            for h in range(1, HT):
                wt = wpool.tile([128, KT, 3, 128], BF, tag=f"w16h_{h}",
                                name=f"w16h_{h}")
                nc.sync.dma_start(wt[:], w16h_d[h][:])
                w16h_t[h] = wt

            a8_t = [None] * KP
            for kp in range(KP):
                at = apool.tile([128, 2, B], E4, tag=f"a8_{kp}", name=f"a8_{kp}")
                nc.gpsimd.dma_start(at[:], a8_d[kp][:])
                a8_t[kp] = at

            a16_t = [None] * KT
            cp_t = [None] * HT
            def _load_a16(k):
                t = apool.tile([128, B], BF, tag=f"a16_{k}", name=f"a16_{k}")
                nc.gpsimd.dma_start(t[:], a16[k * 128:(k + 1) * 128, :])
                a16_t[k] = t
            def _load_cp(h):
                t = apool.tile([128, B], BF, tag=f"cp_{h}", name=f"cp_{h}")
                nc.gpsimd.dma_start(t[:], cp16[h * 128:(h + 1) * 128, :])
                cp_t[h] = t
            for k in range(3):
                _load_a16(k)
            _load_cp(0)
            _load_a16(3)
            _load_a16(4)
            _load_cp(1)
            _load_a16(5)
            _load_a16(6)
            _load_cp(2)
            _load_a16(7)
            _load_cp(3)

            def _w16(k, h, gi):
                if h == 0:
                    return w16h0_t[:, k, gi, :]
                return w16h_t[h][:, k, gi, :]

            # --- phase 1: all i-gate fp8 DoubleRow matmuls ---------------
            # Results parked as f32 SBUF tiles. Pass A: h={0,1,2} x b2
            # kp-major over 6 banks (the ramping PE trails a8 chunk
            # arrivals); pass B: h=3 from SBUF on 2 more banks. Bank tags
            # are shared with phase 2; their phase-2 first use lands well
            # after the matching i-ACT.
            i_t = [[None] * BT for _ in range(HT)]

            def _mm_i(ps, kp, h, b2):
                nc.tensor.matmul(
                    ps[:],
                    w8_t[:, kp, h],
                    a8_t[kp][:, :, b2 * 512:(b2 + 1) * 512],
                    start=(kp == 0), stop=(kp == KP - 1),
                    perf_mode=DR,
                )

            def _act_i(ps, h, b2):
                t = ipool.tile([128, 512], F32, tag=f"i_{h}_{b2}", name=f"i_{h}_{b2}")
                nc.scalar.activation(
                    t[:], ps[:], _SIG,
                    bias=bias_t[:, h:h + 1],  # gate 0 cols
                    scale=1.0 / SW,
                )
                i_t[h][b2] = t

            passA = [(0, 0), (0, 1), (1, 0), (1, 1), (2, 0), (2, 1)]
            tagsA = ["psB0", "psC0", "psB1", "psB2", "psC2", "psD0"]
            psA = {hb: pspool.tile([128, 512], F32, tag=tg, name=f"psI{tg}")
                   for hb, tg in zip(passA, tagsA)}
            for kp in range(KP):
                for (h, b2) in passA:
                    _mm_i(psA[(h, b2)], kp, h, b2)
            for (h, b2) in passA:
                _act_i(psA[(h, b2)], h, b2)
            tagsB = ["psD1", "psC1"]
            for b2 in range(BT):
                ps = pspool.tile([128, 512], F32, tag=tagsB[b2], name=f"psI3{b2}")
                for kp in range(KP):
                    _mm_i(ps, kp, 3, b2)
                _act_i(ps, 3, b2)

            # --- phase 2: pure bf16 (c, f, o) + tails --------------------
            def _ps_g(setname, gi):
                return pspool.tile([128, 512], F32, tag=f"ps{setname}{gi}",
                                   name=f"ps{setname}{gi}")

            def _mm_g(ps, gi, k, h, b2, lo=0, w=512):
                nc.tensor.matmul(
                    ps[:, lo:lo + w],
                    _w16(k, h, gi),
                    a16_t[k][:, b2 * 512 + lo:b2 * 512 + lo + w],
                    start=(k == 0), stop=(k == KT - 1),
                )

            def _tail(h, b2, psum):
                """psum = [c, f, o] banks; i comes from i_t[h][b2]."""
                hs = slice(h * 128, (h + 1) * 128)
                cs = slice(b2 * 512, (b2 + 1) * 512)

                def _act(gi, fn, gname):
                    t = gpool.tile([128, 512], F32, tag=f"g{gname}",
                                   name=f"g{gname}_{h}_{b2}")
                    # bias col: gate order (i, c, f, o) -> 1 + gi
                    nc.scalar.activation(
                        t[:], psum[gi][:], fn,
                        bias=bias_t[:, (1 + gi) * HT + h:(1 + gi) * HT + h + 1],
                    )
                    return t

                gc = _act(0, _TANH, "c")
                gf = _act(1, _SIG, "f")

                t1 = epool.tile([128, 512], F32, tag="t1", name=f"t1_{h}_{b2}")
                nc.vector.tensor_mul(t1[:], i_t[h][b2][:], gc[:])
                t2 = epool.tile([128, 512], F32, tag="t2", name=f"t2_{h}_{b2}")
                nc.vector.tensor_mul(t2[:], gf[:], cp_t[h][:, cs])
                cn = epool.tile([128, 512], BF, tag="cn", name=f"cn_{h}_{b2}")
                nc.vector.tensor_add(cn[:], t1[:], t2[:])
                nc.gpsimd.dma_start(c_nextT[hs, cs], cn[:])

                th = epool.tile([128, 512], BF, tag="th", name=f"th_{h}_{b2}")
                nc.scalar.activation(th[:], cn[:], _TANH)

                go = _act(2, _SIG, "o")
                hn = epool.tile([128, 512], BF, tag="hn", name=f"hn_{h}_{b2}")
                nc.vector.tensor_mul(hn[:], go[:], th[:])
                nc.gpsimd.dma_start(h_nextT[hs, cs], hn[:])

            # h=0: both batch halves k-major (6 banks).
            setname = {0: "B", 1: "C"}
            psum0 = {b2: [_ps_g(setname[b2], gi) for gi in range(3)] for b2 in range(BT)}
            for k in range(KT):
                for gi in range(3):
                    for b2 in range(BT):
                        _mm_g(psum0[b2][gi], gi, k, 0, b2)
            for b2 in range(BT):
                _tail(0, b2, psum0[b2])

            # h in {1, 2}: sequential (h, b2) groups, bank set by batch half.
            for h in (1, 2):
                for b2 in range(BT):
                    psum = [_ps_g(setname[b2], gi) for gi in range(3)]
                    for gi in range(3):
                        for k in range(KT):
                            _mm_g(psum[gi], gi, k, h, b2)
                    _tail(h, b2, psum)

            # --- final h-strip (h=3): tail-minimal ordering --------------
            # Gate-run order: c(b0), c(b1), f(b1), f(b0), o(b0), o(b1) in
            # column halves. f(b1) runs BEFORE f(b0) so b2=1's cn/tanh
            # chain clears the scalar engine early; after the very last
            # matmul only ACT(o half) + mul + DMA remain.
            h = HT - 1
            hs = slice(h * 128, (h + 1) * 128)
            pcf = {b2: [_ps_g(setname[b2], gi) for gi in range(2)]
                   for b2 in range(BT)}
            for b2 in (0, 1):
                for k in range(KT):
                    _mm_g(pcf[b2][0], 0, k, h, b2)
            for b2 in (1, 0):
                for k in range(KT):
                    _mm_g(pcf[b2][1], 1, k, h, b2)

            # scalar: gc0, gc1, gf1, gf0, th10, th11, th00, th01, go0,
            # go1a, go1b -- each ready slightly before its slot.
            gc_, gf_, t1_, t2_, cn_ = {}, {}, {}, {}, {}
            for b2 in (0, 1):
                t = gpool.tile([128, 512], F32, tag=f"ggc{b2}", name=f"gc3_{b2}")
                nc.scalar.activation(
                    t[:], pcf[b2][0][:], _TANH,
                    bias=bias_t[:, 1 * HT + h:1 * HT + h + 1])
                gc_[b2] = t
            for b2 in (1, 0):
                t = gpool.tile([128, 512], F32, tag=f"ggf{b2}", name=f"gf3_{b2}")
                nc.scalar.activation(
                    t[:], pcf[b2][1][:], _SIG,
                    bias=bias_t[:, 2 * HT + h:2 * HT + h + 1])
                gf_[b2] = t
            for b2 in (1, 0):
                cs = slice(b2 * 512, (b2 + 1) * 512)
                t1 = epool.tile([128, 512], F32, tag="t1", name=f"t1_3{b2}")
                nc.vector.tensor_mul(t1[:], i_t[h][b2][:], gc_[b2][:])
                t2 = epool.tile([128, 512], F32, tag="t2", name=f"t2_3{b2}")
                eng = nc.gpsimd if b2 == 1 else nc.vector
                eng.tensor_mul(t2[:], gf_[b2][:], cp_t[h][:, cs])
                cn = epool.tile([128, 512], BF, tag=f"cn3{b2}", name=f"cn3_{b2}")
                nc.vector.tensor_add(cn[:], t1[:], t2[:])
                nc.gpsimd.dma_start(c_nextT[hs, cs], cn[:])
                cn_[b2] = cn
            th_h = {}
            for b2 in (1, 0):
                for ci in range(2):
                    th = epool.tile([128, 256], BF, tag=f"th3{b2}{ci}",
                                    name=f"th3_{b2}{ci}")
                    nc.scalar.activation(
                        th[:], cn_[b2][:, ci * 256:(ci + 1) * 256], _TANH)
                    th_h[(b2, ci)] = th

            # o-gate b2=0: full width; tail overlaps b2=1's o matmuls.
            po0 = _ps_g("B", 2)
            for k in range(KT):
                _mm_g(po0, 2, k, h, 0)
            go0 = gpool.tile([128, 512], F32, tag="ggo0", name="go3_0")
            nc.scalar.activation(
                go0[:], po0[:], _SIG,
                bias=bias_t[:, 3 * HT + h:3 * HT + h + 1])
            hn0 = epool.tile([128, 512], BF, tag="hn30", name="hn3_0")
            nc.vector.tensor_mul(hn0[:, 0:256], go0[:, 0:256], th_h[(0, 0)][:])
            nc.vector.tensor_mul(hn0[:, 256:512], go0[:, 256:512], th_h[(0, 1)][:])
            nc.gpsimd.dma_start(h_nextT[hs, 0:512], hn0[:])

            # o-gate b2=1: column halves; after the last matmul only
            # ACT(o half 1) + mul + DMA remain.
            po1 = _ps_g("C", 2)
            for ci in range(2):
                lo = ci * 256
                for k in range(KT):
                    _mm_g(po1, 2, k, h, 1, lo=lo, w=256)
            for ci in range(2):
                lo = ci * 256
                go = gpool.tile([128, 256], F32, tag=f"ggo1{ci}",
                                name=f"go3_1{ci}")
                nc.scalar.activation(
                    go[:], po1[:, lo:lo + 256], _SIG,
                    bias=bias_t[:, 3 * HT + h:3 * HT + h + 1])
                hn = epool.tile([128, 256], BF, tag=f"hn31{ci}",
                                name=f"hn3_1{ci}")
                nc.vector.tensor_mul(hn[:], go[:], th_h[(1, ci)][:])
                eng = nc.scalar if ci == 0 else nc.sync
                eng.dma_start(h_nextT[hs, 512 + lo:512 + lo + 256], hn[:])

    nc.compile()
    return nc


_NC_CACHE = None
_LAST_IN_MAPS = None


def kernel(x, h_prev, c_prev, W_i, b_i, W_f, b_f, W_c, b_c, W_o, b_o):
    global _NC_CACHE, _LAST_IN_MAPS
    if _NC_CACHE is None:
        _NC_CACHE = _build()
    nc = _NC_CACHE

    np_e4 = mybir.dt.np(E4)
    np_bf = mybir.dt.np(BF)

    combT = np.concatenate([x, h_prev], axis=1).T          # [K, BATCH] f32
    a8_full = combT.astype(np_e4)
    a16_full = combT.astype(np_bf)

    # w8i[p, kp*1024+h*256+j*128+m] = (W_i*SW)[h*128+m, kp*256+j*128+p]
    w8i = np.ascontiguousarray(
        (W_i * SW).astype(np_e4)
        .reshape(HT, 128, KP, 2, 128)      # [h, m, kp, j, p]
        .transpose(4, 2, 0, 3, 1)          # [p, kp, h, j, m]
        .reshape(128, KP * HT * 256)
    )
    # w16[p, h*3072+k*384+gi*128+m] = W_g[h*128+m, k*128+p], gi order (c, f, o)
    w16 = np.ascontiguousarray(
        np.stack([W_c, W_f, W_o])
        .astype(np_bf)
        .reshape(3, HT, 128, KT, 128)      # [gi, h, m, k, p]
        .transpose(4, 1, 3, 0, 2)          # [p, h, k, gi, m]
        .reshape(128, HT * KT * 384)
    )
    # bias2d[m, g*HT+h] = b_g[h*128+m], gate order (i, c, f, o)
    bias2d = np.ascontiguousarray(
        np.stack([b_i, b_c, b_f, b_o])
        .reshape(4, HT, 128)
        .transpose(2, 0, 1)
        .reshape(128, 4 * HT)
    ).astype(np.float32)
    cp_full = c_prev.T.astype(np_bf)                       # [H, BATCH]

    in_maps = []
    for j in range(N_CORES):
        cols = slice(j * B, (j + 1) * B)
        a8_core = (
            a8_full[:, cols].reshape(KP, 2, 128, B)       # [kp, j2, p, b]
            .transpose(2, 0, 1, 3)                        # [p, kp, j2, b]
        )
        im = {
            "w8i": w8i,
            "a16": np.ascontiguousarray(a16_full[:, cols]),
            "w16h0": np.ascontiguousarray(w16[:, 0:3072]),
            "bias2d": bias2d,
            "cp16": np.ascontiguousarray(cp_full[:, cols]),
        }
        for kp in range(KP):
            im[f"a8_{kp}"] = np.ascontiguousarray(
                a8_core[:, kp].reshape(128, 2 * B))
        for h in range(1, HT):
            im[f"w16h_{h}"] = np.ascontiguousarray(
                w16[:, h * 3072:(h + 1) * 3072])
        in_maps.append(im)

    _LAST_IN_MAPS = in_maps
    try:
        res = run_bass_kernel_spmd(nc, in_maps, core_ids=list(range(N_CORES)))
    except Exception:
        # transient NRT_EXEC_UNIT_UNRECOVERABLE has been observed once on an
        # otherwise-correct NEFF; one retry is cheap insurance.
        res = run_bass_kernel_spmd(nc, in_maps, core_ids=list(range(N_CORES)))

    h_next = np.concatenate([r["h_nextT"].T for r in res.results], axis=0)
    c_next = np.concatenate([r["c_nextT"].T for r in res.results], axis=0)
    return (h_next.astype(np.float32), c_next.astype(np.float32))


# revision 9
# speedup vs baseline: 1.0155x; 1.0155x over previous
"""LSTM cell (batch 8192, input 512, hidden 512) on 8 Trainium2 NeuronCores.

Data-parallel over the batch dim: each core handles 1024 rows; weights are
replicated. Everything is computed in [hidden, batch] layout with the
contraction dim (fan_in = 1024) on SBUF partitions:

  gate.T[n, b] = sum_k W.T[k, n] * combined.T[k, b]     (matmul: lhsT.T @ rhs)

Precision plan (measured rel-err ~1.55e-2 vs the 2e-2 budget): the i-gate
matmul runs in fp8-e4m3 with MatmulPerfMode.DoubleRow (K=256 per
instruction at the same ~216ns fill as a K=128 bf16 matmul = 2x MACs),
weights pre-scaled by 128 and descaled via the ACT scale operand. The
f/c/o gates stay bf16: their error amplification (f multiplies c_prev,
c~ passes tanh' = 1, o hits h directly) makes fp8 too lossy for them.
Gates are f32; cn/th/hn and both outputs are bf16; c_prev ships bf16.

Schedule facts this version is built around (measured on HW + trace):
- Matmul fill is N columns at 2.4GHz regardless of dtype: 512-col matmul
  = ~216ns back-to-back. PE floor = 224 matmul-equivalents x 216 ~ 48.4us.
- PE clock ramps to full over ~4-6us of near-continuous activity (sub-us
  gaps don't reset it); warmup fp8-DR dummies run straight into phase 1.
- DMA per-queue throughput is PACKET-CADENCE bound (~160-400ns/packet,
  packets = contiguous runs capped at 4KB). Tensors with short rows
  (<2KB) crawl at <100GB/s; >=4KB rows hit ~350GB/s. So w8i ships as ONE
  [128,4096] tensor (sync ring) while a8/a16/cp chunk into 256KB pieces
  with 2KB rows (gpsimd ring) for early semaphores at decent rate.
- Phase 1 (i-gate, fp8) runs kp-major over h={0,1,2} x b2 (6 PSUM banks)
  then h=3 from SBUF on 2 more banks; the ramping PE (~430ns/mm) trails
  the a8 chunk arrivals (~1us apart), so no stalls.
- The measured window ends after the LAST output DMA packet + drains, so
  the final h-strip computes c for both halves, then f for b2=1 BEFORE
  b2=0 (so b2=1's cn/tanh chain clears the scalar engine early), then o
  for b2=0, then o for b2=1 in column halves: after the very last matmul
  only ACT(o half) + mul + DMA remain. Final chunks fan out across the
  gpsimd/scalar/sync rings to overlap the ~1us DGE latency.
- ~10.8us of the measured window is fixed framework pre/postamble (const
  memsets, engine preambles, 254 semaphore clears at exit) - not
  reachable from kernel code.
"""

import numpy as np

import concourse.bacc as bacc
import concourse.bass as bass
import concourse.mybir as mybir
from concourse import tile
from concourse.bass_utils import run_bass_kernel_spmd

N_CORES = 8
BATCH = 8192
B = BATCH // N_CORES  # 1024 batch rows per core
K = 1024              # fan_in = input_dim + hidden_dim
H = 512               # hidden dim
KT = K // 128         # 8 bf16 contraction tiles
KP = K // 256         # 4 fp8 DoubleRow contraction tiles
HT = H // 128         # 4 hidden chunks per gate
BT = B // 512         # 2 batch halves (PSUM free-dim limit is 512 f32)
SW = 128.0            # fp8 weight pre-scale (descaled in ACT)
NWARM = 6             # fp8-DR warmup matmuls (cover preamble->w8i lands)

E4 = mybir.dt.float8e4
BF = mybir.dt.bfloat16
F32 = mybir.dt.float32
DR = mybir.MatmulPerfMode.DoubleRow

_SIG = mybir.ActivationFunctionType.Sigmoid
_TANH = mybir.ActivationFunctionType.Tanh


def _build():
    nc = bacc.Bacc(
        "TRN2",
        target_bir_lowering=False,
        debug=False,
        num_devices=N_CORES,
    )

    # a8_{kp} rows p, cols j*B + b  (j = which 128-half of the 256 k-block)
    a8_d = [nc.dram_tensor(f"a8_{kp}", [128, 2 * B], E4, kind="ExternalInput")
            for kp in range(KP)]
    # w8i rows p, cols kp*1024 + h*256 + j*128 + m  (4KB rows -> fast DMA)
    w8i = nc.dram_tensor("w8i", [128, KP * HT * 256], E4, kind="ExternalInput")
    # a16 rows k*128+p, cols b  ([128,B] slices are contiguous 256KB)
    a16 = nc.dram_tensor("a16", [K, B], BF, kind="ExternalInput")
    # w16h0 rows p, cols k*384 + gi*128 + m  (gi: 0=c, 1=f, 2=o; 6KB rows)
    w16h0 = nc.dram_tensor("w16h0", [128, KT * 384], BF, kind="ExternalInput")
    # w16h_{h} rows p, cols k*384 + gi*128 + m
    w16h_d = [None] + [
        nc.dram_tensor(f"w16h_{h}", [128, KT * 384], BF, kind="ExternalInput")
        for h in range(1, HT)]
    # bias2d col = g*HT + h, gate order (i, c, f, o)
    bias2d = nc.dram_tensor("bias2d", [128, 4 * HT], F32, kind="ExternalInput")
    cp16 = nc.dram_tensor("cp16", [H, B], BF, kind="ExternalInput")
    h_nextT = nc.dram_tensor("h_nextT", [H, B], BF, kind="ExternalOutput")
    c_nextT = nc.dram_tensor("c_nextT", [H, B], BF, kind="ExternalOutput")

    with tile.TileContext(nc) as tc:
        with (
            tc.tile_pool(name="acts", bufs=1) as apool,
            tc.tile_pool(name="wts", bufs=1) as wpool,
            tc.tile_pool(name="igates", bufs=1) as ipool,
            tc.tile_pool(name="gates", bufs=3) as gpool,
            tc.tile_pool(name="ew", bufs=3) as epool,
            tc.tile_pool(name="psum", bufs=1, space="PSUM") as pspool,
        ):
            # --- PE clock warm-up ----------------------------------------
            # fp8-DR dummies over a memset tile while the first input DMAs
            # are in flight; same dtype as phase 1, sized to end right as
            # w8i lands so the PE never idles (idle resets the DVFS ramp).
            warm_t = wpool.tile([128, 2, 512], E4, tag="warm", name="warm")
            nc.vector.memset(warm_t[:], 0.0)
            ps_warm = pspool.tile([128, 512], F32, tag="psC1", name="ps_warm")
            for r in range(NWARM):
                nc.tensor.matmul(
                    ps_warm[:], warm_t[:, :, 0:128], warm_t[:],
                    start=(r == 0), stop=(r == NWARM - 1),
                    perf_mode=DR,
                )
            warm_o = wpool.tile([128, 512], F32, tag="warm_o", name="warm_o")
            nc.vector.tensor_scalar_mul(warm_o[:], ps_warm[:], 0.0)

            # --- input DMA: two rings, exact need order -------------------
            # sync: bias, w8i, then the four fat w16 strips (all 4-6KB
            # rows -> full-rate); gpsimd: the 256KB activation chunks
            # (2KB rows) whose early semaphores pace phases 1 and 2.
            bias_t = wpool.tile([128, 4 * HT], F32, tag="bias", name="bias")
            nc.sync.dma_start(bias_t[:], bias2d[:])

            w8_t = wpool.tile([128, KP, HT, 2, 128], E4, tag="w8", name="w8")
            nc.sync.dma_start(w8_t[:], w8i[:])

            a8_t = [None] * KP
            for kp in range(KP):
                at = apool.tile([128, 2, B], E4, tag=f"a8_{kp}", name=f"a8_{kp}")
                nc.sync.dma_start(at[:], a8_d[kp][:])
                a8_t[kp] = at

            w16h0_t = wpool.tile([128, KT, 3, 128], BF, tag="w16h0", name="w16h0")
            nc.sync.dma_start(w16h0_t[:], w16h0[:])
            a16_t = [None] * KT
            for k in range(KT):
                t = apool.tile([128, B], BF, tag=f"a16_{k}", name=f"a16_{k}")
                nc.sync.dma_start(t[:], a16[k * 128:(k + 1) * 128, :])
                a16_t[k] = t

            # scalar ring (idle after its act-table loads): the fat h>=1
            # weight strips + c_prev chunks, all first needed >=20us.
            w16h_t = [None] * HT
            cp_t = [None] * HT
            def _load_w16h(h):
                wt = wpool.tile([128, KT, 3, 128], BF, tag=f"w16h_{h}",
                                name=f"w16h_{h}")
                nc.scalar.dma_start(wt[:], w16h_d[h][:])
                w16h_t[h] = wt
            def _load_cp(h):
                t = apool.tile([128, B], BF, tag=f"cp_{h}", name=f"cp_{h}")
                nc.scalar.dma_start(t[:], cp16[h * 128:(h + 1) * 128, :])
                cp_t[h] = t
            _load_w16h(1)
            _load_cp(0)
            _load_w16h(2)
            _load_cp(1)
            _load_w16h(3)
            _load_cp(2)
            _load_cp(3)

            def _w16(k, h, gi):
                if h == 0:
                    return w16h0_t[:, k, gi, :]
                return w16h_t[h][:, k, gi, :]

            # --- phase 1: all i-gate fp8 DoubleRow matmuls ---------------
            # Results parked as f32 SBUF tiles. Pass A: h={0,1,2} x b2
            # kp-major over 6 banks (the ramping PE trails a8 chunk
            # arrivals); pass B: h=3 from SBUF on 2 more banks. Bank tags
            # are shared with phase 2; their phase-2 first use lands well
            # after the matching i-ACT.
            i_t = [[None] * BT for _ in range(HT)]

            def _mm_i(ps, kp, h, b2):
                nc.tensor.matmul(
                    ps[:],
                    w8_t[:, kp, h],
                    a8_t[kp][:, :, b2 * 512:(b2 + 1) * 512],
                    start=(kp == 0), stop=(kp == KP - 1),
                    perf_mode=DR,
                )

            def _act_i(ps, h, b2):
                t = ipool.tile([128, 512], F32, tag=f"i_{h}_{b2}", name=f"i_{h}_{b2}")
                nc.scalar.activation(
                    t[:], ps[:], _SIG,
                    bias=bias_t[:, h:h + 1],  # gate 0 cols
                    scale=1.0 / SW,
                )
                i_t[h][b2] = t

            passA = [(0, 0), (0, 1), (1, 0), (1, 1), (2, 0), (2, 1)]
            tagsA = ["psB0", "psC0", "psB1", "psB2", "psC2", "psD0"]
            psA = {hb: pspool.tile([128, 512], F32, tag=tg, name=f"psI{tg}")
                   for hb, tg in zip(passA, tagsA)}
            for kp in range(KP):
                for (h, b2) in passA:
                    _mm_i(psA[(h, b2)], kp, h, b2)
            for (h, b2) in passA:
                _act_i(psA[(h, b2)], h, b2)
            tagsB = ["psD1", "psC1"]
            for b2 in range(BT):
                ps = pspool.tile([128, 512], F32, tag=tagsB[b2], name=f"psI3{b2}")
                for kp in range(KP):
                    _mm_i(ps, kp, 3, b2)
                _act_i(ps, 3, b2)

            # --- phase 2: pure bf16 (c, f, o) + tails --------------------
            def _ps_g(setname, gi):
                return pspool.tile([128, 512], F32, tag=f"ps{setname}{gi}",
                                   name=f"ps{setname}{gi}")

            def _mm_g(ps, gi, k, h, b2, lo=0, w=512):
                nc.tensor.matmul(
                    ps[:, lo:lo + w],
                    _w16(k, h, gi),
                    a16_t[k][:, b2 * 512 + lo:b2 * 512 + lo + w],
                    start=(k == 0), stop=(k == KT - 1),
                )

            def _tail(h, b2, psum):
                """psum = [c, f, o] banks; i comes from i_t[h][b2]."""
                hs = slice(h * 128, (h + 1) * 128)
                cs = slice(b2 * 512, (b2 + 1) * 512)

                def _act(gi, fn, gname):
                    t = gpool.tile([128, 512], F32, tag=f"g{gname}",
                                   name=f"g{gname}_{h}_{b2}")
                    # bias col: gate order (i, c, f, o) -> 1 + gi
                    nc.scalar.activation(
                        t[:], psum[gi][:], fn,
                        bias=bias_t[:, (1 + gi) * HT + h:(1 + gi) * HT + h + 1],
                    )
                    return t

                gc = _act(0, _TANH, "c")
                gf = _act(1, _SIG, "f")

                t1 = epool.tile([128, 512], F32, tag="t1", name=f"t1_{h}_{b2}")
                nc.vector.tensor_mul(t1[:], i_t[h][b2][:], gc[:])
                t2 = epool.tile([128, 512], F32, tag="t2", name=f"t2_{h}_{b2}")
                nc.vector.tensor_mul(t2[:], gf[:], cp_t[h][:, cs])
                cn = epool.tile([128, 512], BF, tag="cn", name=f"cn_{h}_{b2}")
                nc.vector.tensor_add(cn[:], t1[:], t2[:])
                nc.gpsimd.dma_start(c_nextT[hs, cs], cn[:])

                th = epool.tile([128, 512], BF, tag="th", name=f"th_{h}_{b2}")
                nc.scalar.activation(th[:], cn[:], _TANH)

                go = _act(2, _SIG, "o")
                hn = epool.tile([128, 512], BF, tag="hn", name=f"hn_{h}_{b2}")
                nc.vector.tensor_mul(hn[:], go[:], th[:])
                nc.gpsimd.dma_start(h_nextT[hs, cs], hn[:])

            # h=0: both batch halves k-major (6 banks).
            setname = {0: "B", 1: "C"}
            psum0 = {b2: [_ps_g(setname[b2], gi) for gi in range(3)] for b2 in range(BT)}
            for k in range(KT):
                for gi in range(3):
                    for b2 in range(BT):
                        _mm_g(psum0[b2][gi], gi, k, 0, b2)
            for b2 in range(BT):
                _tail(0, b2, psum0[b2])

            # h in {1, 2}: sequential (h, b2) groups, bank set by batch half.
            for h in (1, 2):
                for b2 in range(BT):
                    psum = [_ps_g(setname[b2], gi) for gi in range(3)]
                    for gi in range(3):
                        for k in range(KT):
                            _mm_g(psum[gi], gi, k, h, b2)
                    _tail(h, b2, psum)

            # --- final h-strip (h=3): tail-minimal ordering --------------
            # Gate-run order: c(b0), c(b1), f(b1), f(b0), o(b0), o(b1) in
            # column halves. f(b1) runs BEFORE f(b0) so b2=1's cn/tanh
            # chain clears the scalar engine early; after the very last
            # matmul only ACT(o half) + mul + DMA remain.
            h = HT - 1
            hs = slice(h * 128, (h + 1) * 128)
            pcf = {b2: [_ps_g(setname[b2], gi) for gi in range(2)]
                   for b2 in range(BT)}
            for b2 in (0, 1):
                for k in range(KT):
                    _mm_g(pcf[b2][0], 0, k, h, b2)
            for b2 in (1, 0):
                for k in range(KT):
                    _mm_g(pcf[b2][1], 1, k, h, b2)

            # scalar: gc0, gc1, gf1, gf0, th10, th11, th00, th01, go0,
            # go1a, go1b -- each ready slightly before its slot.
            gc_, gf_, t1_, t2_, cn_ = {}, {}, {}, {}, {}
            for b2 in (0, 1):
                t = gpool.tile([128, 512], F32, tag=f"ggc{b2}", name=f"gc3_{b2}")
                nc.scalar.activation(
                    t[:], pcf[b2][0][:], _TANH,
                    bias=bias_t[:, 1 * HT + h:1 * HT + h + 1])
                gc_[b2] = t
            for b2 in (1, 0):
                t = gpool.tile([128, 512], F32, tag=f"ggf{b2}", name=f"gf3_{b2}")
                nc.scalar.activation(
                    t[:], pcf[b2][1][:], _SIG,
                    bias=bias_t[:, 2 * HT + h:2 * HT + h + 1])
                gf_[b2] = t
            for b2 in (1, 0):
                cs = slice(b2 * 512, (b2 + 1) * 512)
                t1 = epool.tile([128, 512], F32, tag="t1", name=f"t1_3{b2}")
                nc.vector.tensor_mul(t1[:], i_t[h][b2][:], gc_[b2][:])
                t2 = epool.tile([128, 512], F32, tag="t2", name=f"t2_3{b2}")
                eng = nc.gpsimd if b2 == 1 else nc.vector
                eng.tensor_mul(t2[:], gf_[b2][:], cp_t[h][:, cs])
                cn = epool.tile([128, 512], BF, tag=f"cn3{b2}", name=f"cn3_{b2}")
                nc.vector.tensor_add(cn[:], t1[:], t2[:])
                nc.gpsimd.dma_start(c_nextT[hs, cs], cn[:])
                cn_[b2] = cn
            th_h = {}
            for b2 in (1, 0):
                for ci in range(2):
                    th = epool.tile([128, 256], BF, tag=f"th3{b2}{ci}",
                                    name=f"th3_{b2}{ci}")
                    nc.scalar.activation(
                        th[:], cn_[b2][:, ci * 256:(ci + 1) * 256], _TANH)
                    th_h[(b2, ci)] = th

            # o-gate b2=0: full width; tail overlaps b2=1's o matmuls.
            po0 = _ps_g("B", 2)
            for k in range(KT):
                _mm_g(po0, 2, k, h, 0)
            go0 = gpool.tile([128, 512], F32, tag="ggo0", name="go3_0")
            nc.scalar.activation(
                go0[:], po0[:], _SIG,
                bias=bias_t[:, 3 * HT + h:3 * HT + h + 1])
            hn0 = epool.tile([128, 512], BF, tag="hn30", name="hn3_0")
            nc.vector.tensor_mul(hn0[:, 0:256], go0[:, 0:256], th_h[(0, 0)][:])
            nc.vector.tensor_mul(hn0[:, 256:512], go0[:, 256:512], th_h[(0, 1)][:])
            nc.gpsimd.dma_start(h_nextT[hs, 0:512], hn0[:])

            # o-gate b2=1: column halves; after the last matmul only
            # ACT(o half 1) + mul + DMA remain.
            po1 = _ps_g("C", 2)
            for ci in range(2):
                lo = ci * 256
                for k in range(KT):
                    _mm_g(po1, 2, k, h, 1, lo=lo, w=256)
            for ci in range(2):
                lo = ci * 256
                go = gpool.tile([128, 256], F32, tag=f"ggo1{ci}",
                                name=f"go3_1{ci}")
                nc.scalar.activation(
                    go[:], po1[:, lo:lo + 256], _SIG,
                    bias=bias_t[:, 3 * HT + h:3 * HT + h + 1])
                hn = epool.tile([128, 256], BF, tag=f"hn31{ci}",
                                name=f"hn3_1{ci}")
                nc.vector.tensor_mul(hn[:], go[:], th_h[(1, ci)][:])
                eng = nc.scalar if ci == 0 else nc.sync
                eng.dma_start(h_nextT[hs, 512 + lo:512 + lo + 256], hn[:])

    nc.compile()
    return nc


_NC_CACHE = None
_LAST_IN_MAPS = None


def kernel(x, h_prev, c_prev, W_i, b_i, W_f, b_f, W_c, b_c, W_o, b_o):
    global _NC_CACHE, _LAST_IN_MAPS
    if _NC_CACHE is None:
        _NC_CACHE = _build()
    nc = _NC_CACHE

    np_e4 = mybir.dt.np(E4)
    np_bf = mybir.dt.np(BF)

    combT = np.concatenate([x, h_prev], axis=1).T          # [K, BATCH] f32
    a8_full = combT.astype(np_e4)
    a16_full = combT.astype(np_bf)

    # w8i[p, kp*1024+h*256+j*128+m] = (W_i*SW)[h*128+m, kp*256+j*128+p]
    w8i = np.ascontiguousarray(
        (W_i * SW).astype(np_e4)
        .reshape(HT, 128, KP, 2, 128)      # [h, m, kp, j, p]
        .transpose(4, 2, 0, 3, 1)          # [p, kp, h, j, m]
        .reshape(128, KP * HT * 256)
    )
    # w16[p, h*3072+k*384+gi*128+m] = W_g[h*128+m, k*128+p], gi order (c, f, o)
    w16 = np.ascontiguousarray(
        np.stack([W_c, W_f, W_o])
        .astype(np_bf)
        .reshape(3, HT, 128, KT, 128)      # [gi, h, m, k, p]
        .transpose(4, 1, 3, 0, 2)          # [p, h, k, gi, m]
        .reshape(128, HT * KT * 384)
    )
    # bias2d[m, g*HT+h] = b_g[h*128+m], gate order (i, c, f, o)
    bias2d = np.ascontiguousarray(
        np.stack([b_i, b_c, b_f, b_o])
        .reshape(4, HT, 128)
        .transpose(2, 0, 1)
        .reshape(128, 4 * HT)
    ).astype(np.float32)
    cp_full = c_prev.T.astype(np_bf)                       # [H, BATCH]

    in_maps = []
    for j in range(N_CORES):
        cols = slice(j * B, (j + 1) * B)
        a8_core = (
            a8_full[:, cols].reshape(KP, 2, 128, B)       # [kp, j2, p, b]
            .transpose(2, 0, 1, 3)                        # [p, kp, j2, b]
        )
        im = {
            "w8i": w8i,
            "a16": np.ascontiguousarray(a16_full[:, cols]),
            "w16h0": np.ascontiguousarray(w16[:, 0:3072]),
            "bias2d": bias2d,
            "cp16": np.ascontiguousarray(cp_full[:, cols]),
        }
        for kp in range(KP):
            im[f"a8_{kp}"] = np.ascontiguousarray(
                a8_core[:, kp].reshape(128, 2 * B))
        for h in range(1, HT):
            im[f"w16h_{h}"] = np.ascontiguousarray(
                w16[:, h * 3072:(h + 1) * 3072])
        in_maps.append(im)

    _LAST_IN_MAPS = in_maps
    try:
        res = run_bass_kernel_spmd(nc, in_maps, core_ids=list(range(N_CORES)))
    except Exception:
        # transient NRT_EXEC_UNIT_UNRECOVERABLE has been observed once on an
        # otherwise-correct NEFF; one retry is cheap insurance.
        res = run_bass_kernel_spmd(nc, in_maps, core_ids=list(range(N_CORES)))

    h_next = np.concatenate([r["h_nextT"].T for r in res.results], axis=0)
    c_next = np.concatenate([r["c_nextT"].T for r in res.results], axis=0)
    return (h_next.astype(np.float32), c_next.astype(np.float32))
